# revision 53
# baseline (speedup 1.0000x reference)
"""GrapherModule (dynamic-KNN GAT block) as a hand-written Bass/Tile kernel
for 8 Trainium2 NeuronCores.

Sharding: 8 shards = 4 images x 2 destination-node halves (data parallel,
no collectives). Each core receives its image's node features rotated so
that its 512 destination nodes sit at positions 0..511; all 1024 nodes are
kept as gather sources. The KNN graph (top-16 by similarity), the 4-head
GAT attention and the aggregation are computed with dense masked matmuls:

  y   = BN1(x @ W1.T + b1)                      (fp32, folded BN)
  Sp  = y_dest @ y.T - 0.5*||y_m||^2            (fp32, PE)
  t16 = 16th largest per row (DVE max8 + match_replace + max8)
  mask= -150 where Sp < t16 else 0              (additive pre-leakyrelu)
  h'  = y @ (Wg * bn_scale)                     (bf16, PE)
  w   = exp(leaky_relu(a_src[m] + a_dst[dest] + mask))   (ACT)
  g   = sum_m w * h'[m] / (4 * sum_m w) + cst   (PE accumulated, per head)
  out = BN2(gelu(g) @ W2f.T + b2f) + x          (bf16 matmul + fp32 add)

The whole forward runs as one NEFF; the host only folds BN params,
rotates/slices inputs, and reassembles the output.
"""

import numpy as np

B, C, H, W = 4, 192, 32, 32
N = H * W           # 1024 nodes
ND = N // 2         # 512 destination nodes per core
K = 16
HEADS = 4
HD = 384
BN_EPS = 1e-5
NCORES = 8
MASK_NEG = -150.0
REPL_NEG = -1.0e30

_CC = [(0, 128), (128, 64)]   # contraction chunks for C=192


DEBUG_TAPS = False


def _emit(nc, tc, t):
    """Emit the per-core program. t: dict of dram APs."""
    from contextlib import ExitStack

    import concourse.bass as bass
    import concourse.mybir as mybir
    from concourse.masks import make_identity

    f32 = mybir.dt.float32
    bf16 = mybir.dt.bfloat16
    Alu = mybir.AluOpType
    Act = mybir.ActivationFunctionType

    ctx = ExitStack()
    const = ctx.enter_context(tc.tile_pool(name="const", bufs=1))
    scr = ctx.enter_context(tc.tile_pool(name="scr", bufs=2))
    m8p = ctx.enter_context(tc.tile_pool(name="m8", bufs=4))
    ep = ctx.enter_context(tc.tile_pool(name="ep", bufs=4))
    rzp = ctx.enter_context(tc.tile_pool(name="rz", bufs=8))
    pt = ctx.enter_context(tc.tile_pool(name="pt", bufs=4, space="PSUM"))
    pg = ctx.enter_context(tc.tile_pool(name="pg", bufs=4, space="PSUM"))

    def ctile(shape, dtype, tag):
        return const.tile(shape, dtype, tag=tag, name=tag)

    # ---- constants / inputs in SBUF ----
    f32r = mybir.dt.float32r
    x0 = ctile([128, N], f32r, "x0")
    x1 = ctile([64, N], f32r, "x1")
    for nh in range(2):
        sl = slice(nh * 512, (nh + 1) * 512)
        nc.sync.dma_start(x0[:, sl], t["x"][0:128, sl].bitcast(f32r))
        nc.sync.dma_start(x1[:, sl], t["x"][128:192, sl].bitcast(f32r))

    Wf0 = ctile([128, C], f32r, "Wf0")
    Wf1 = ctile([64, C], f32r, "Wf1")
    nc.sync.dma_start(Wf0, t["W1fT"][0:128, :].bitcast(f32r))
    nc.sync.dma_start(Wf1, t["W1fT"][128:192, :].bitcast(f32r))
    b1f0 = ctile([128, 1], f32, "b1f0")
    b1f1 = ctile([64, 1], f32, "b1f1")
    nc.sync.dma_start(b1f0, t["b1f"][0:128, :])
    nc.sync.dma_start(b1f1, t["b1f"][128:192, :])

    V0 = ctile([128, 8], f32r, "V0")
    V1 = ctile([64, 8], f32r, "V1")
    nc.sync.dma_start(V0, t["V"][0:128, :].bitcast(f32r))
    nc.sync.dma_start(V1, t["V"][128:192, :].bitcast(f32r))

    Wg0 = ctile([128, HEADS * HD], bf16, "Wg0")
    Wg1 = ctile([64, HEADS * HD], bf16, "Wg1")
    nc.sync.dma_start(Wg0, t["WgTs"][0:128, :])
    nc.sync.dma_start(Wg1, t["WgTs"][128:192, :])

    Wt = []
    for i in range(3):
        w_ = ctile([128, C], bf16, f"Wt{i}")
        nc.sync.dma_start(w_, t["W2fT"][i * 128:(i + 1) * 128, :])
        Wt.append(w_)
    b2f0 = ctile([128, 1], f32, "b2f0")
    b2f1 = ctile([64, 1], f32, "b2f1")
    nc.sync.dma_start(b2f0, t["b2f"][0:128, :])
    nc.sync.dma_start(b2f1, t["b2f"][128:192, :])
    cst_row = ctile([1, HD], f32, "cst_row")
    nc.sync.dma_start(cst_row, t["cst"])

    identb = ctile([128, 128], bf16, "identb")
    make_identity(nc, identb)
    ones_row = ctile([1, 128], f32, "ones_row")
    nc.vector.memset(ones_row, 1.0)

    # ---- P1: y = x @ W1f.T + b1f  -> yT [cout, n] fp32 (+ bf16 copy) ----
    # yT rows 128..191 live in y1e[0:64]; y1e row 64 = ones and y1m row 64 =
    # -0.5*||y_m||^2 so the similarity bias rides the second matmul chunk.
    yT0 = ctile([128, N], f32, "yT0")
    y1e = ctile([65, N], f32, "y1e")
    y1m = ctile([65, N], f32, "y1m")
    yb0 = ctile([128, N], bf16, "yb0")
    yb1 = ctile([64, N], bf16, "yb1")
    yT1 = y1e[0:64, :]
    for ct, (c0, cl) in enumerate(_CC):
        for nh in range(2):
            ps = pt.tile([cl, 512], f32, tag="t")
            nc.tensor.matmul(ps, Wf0[:, c0:c0 + cl], x0[:, nh * 512:(nh + 1) * 512],
                             start=True, stop=False)
            nc.tensor.matmul(ps, Wf1[:, c0:c0 + cl], x1[:, nh * 512:(nh + 1) * 512],
                             start=False, stop=True)
            bf = (b1f0, b1f1)[ct]
            f32r_ = mybir.dt.float32r
            if ct == 0:
                nc.vector.tensor_scalar_add(
                    yT0[:, nh * 512:(nh + 1) * 512].bitcast(f32r_), ps, bf)
            else:
                nc.vector.tensor_scalar_add(
                    y1e[0:64, nh * 512:(nh + 1) * 512].bitcast(f32r_), ps, bf[0:64])
    ones_rowN = ctile([1, N], f32, "ones_rowN")
    nc.vector.memset(ones_rowN, 1.0)
    nc.vector.tensor_copy(y1e[64:65, :].bitcast(mybir.dt.float32r), ones_rowN)
    nc.any.tensor_copy(y1m[0:64, :].bitcast(mybir.dt.float32r), y1e[0:64, :])
    nc.vector.tensor_copy(yb0, yT0)
    nc.vector.tensor_copy(yb1, y1e[0:64, :])

    # ---- P2: msq[m] = -0.5 * sum_c y^2 ----
    ysq0 = scr.tile([128, N], f32, tag="scr")
    nc.scalar.activation(ysq0.bitcast(f32r), yT0, Act.Square)
    ysq1 = scr.tile([64, N], f32, tag="scr1")
    nc.scalar.activation(ysq1.bitcast(f32r), yT1, Act.Square)
    ocs = ctile([128, 1], f32, "ocs")
    nc.vector.memset(ocs, 1.0)
    onescol0 = ctile([128, 1], f32r, "oc0")
    onescol1 = ctile([64, 1], f32r, "oc1")
    nc.vector.tensor_copy(onescol0, ocs)
    nc.vector.tensor_copy(onescol1, ocs[0:64, :])
    for nh in range(2):
        ps = pt.tile([1, 512], f32, tag="t")
        nc.tensor.matmul(ps, onescol0, ysq0[:, nh * 512:(nh + 1) * 512].bitcast(f32r),
                         start=True, stop=False)
        nc.tensor.matmul(ps, onescol1, ysq1[:, nh * 512:(nh + 1) * 512].bitcast(f32r),
                         start=False, stop=True)
        nc.scalar.activation(y1m[64:65, nh * 512:(nh + 1) * 512].bitcast(f32r),
                             ps, Act.Copy, scale=-0.5)

    # ---- P3: Sp, top-16 threshold, additive mask ----
    maskneg = [ctile([128, N], bf16, f"mn{dt}") for dt in range(4)]
    for dt in range(4):
        S_sb = scr.tile([128, N], f32, tag="S")
        for nh in range(2):
            ps = pt.tile([128, 512], f32, tag="t")
            nc.tensor.matmul(ps, yT0[:, dt * 128:(dt + 1) * 128].bitcast(f32r),
                             yT0[:, nh * 512:(nh + 1) * 512].bitcast(f32r),
                             start=True, stop=False)
            nc.tensor.matmul(ps, y1e[:, dt * 128:(dt + 1) * 128].bitcast(f32r),
                             y1m[:, nh * 512:(nh + 1) * 512].bitcast(f32r),
                             start=False, stop=True)
            nc.scalar.copy(S_sb[:, nh * 512:(nh + 1) * 512], ps)
        m8a = m8p.tile([128, 8], f32, tag="m8a")
        nc.vector.max(out=m8a, in_=S_sb)
        S_rep = scr.tile([128, N], f32, tag="srep")
        nc.vector.match_replace(out=S_rep, in_to_replace=m8a, in_values=S_sb,
                                imm_value=REPL_NEG)
        m8b = m8p.tile([128, 8], f32, tag="m8b")
        nc.vector.max(out=m8b, in_=S_rep)
        nc.vector.tensor_scalar(out=maskneg[dt], in0=S_sb, scalar1=m8b[:, 7:8],
                                scalar2=MASK_NEG, op0=Alu.is_lt, op1=Alu.mult)

    # ---- P4: a_dst columns [dest, 4] and broadcast a_src planes ----
    ad_sb = ctile([128, 4, 4], f32, "ad_sb")
    for mc in range(4):   # only dest chunks need a_dst
        ps = pt.tile([128, 4], f32, tag="t")
        nc.tensor.matmul(ps, yT0[:, mc * 128:(mc + 1) * 128].bitcast(f32r),
                         V0[:, 4:8], start=True, stop=False)
        nc.tensor.matmul(ps, y1e[0:64, mc * 128:(mc + 1) * 128].bitcast(f32r),
                         V1[:, 4:8], start=False, stop=True)
        nc.vector.tensor_copy(ad_sb[:, mc, :], ps)
    asrcB = []
    for hh in range(4):
        row = ctile([1, N], bf16, f"asrcT{hh}")
        for nh2 in range(2):
            ps = pt.tile([1, 512], f32, tag="t", name=f"psat{hh}_{nh2}")
            nc.tensor.matmul(ps, V0[:, hh:hh + 1],
                             yT0[:, nh2 * 512:(nh2 + 1) * 512].bitcast(f32r),
                             start=True, stop=False)
            nc.tensor.matmul(ps, V1[:, hh:hh + 1],
                             y1e[0:64, nh2 * 512:(nh2 + 1) * 512].bitcast(f32r),
                             start=False, stop=True)
            nc.scalar.copy(row[:, nh2 * 512:(nh2 + 1) * 512], ps)
        nc.sync.dma_start(t["ascr"][hh:hh + 1, :], row)
        ab = ctile([128, N], bf16, f"asrcB{hh}")
        bcast = bass.AP(tensor=t["ascr"].tensor, offset=hh * N,
                        ap=[[0, 128], [1, N]])
        nc.sync.dma_start(ab, bcast)
        asrcB.append(ab)

    # ---- P5: h' = y @ Wg' in bf16, [m, head, 385] with 4.0 in col 384 ----
    h_sb = [ctile([128, HEADS, HD + 1], bf16, f"h{mc}") for mc in range(8)]
    for mc in range(8):
        nc.vector.memset(h_sb[mc][:, :, HD:HD + 1], 4.0)
        for hh in range(4):
            ps = pg.tile([128, HD], f32, tag="g", name=f"psh{mc}_{hh}")
            nc.tensor.matmul(ps, yb0[:, mc * 128:(mc + 1) * 128],
                             Wg0[:, hh * HD:(hh + 1) * HD], start=True, stop=False)
            nc.tensor.matmul(ps, yb1[:, mc * 128:(mc + 1) * 128],
                             Wg1[:, hh * HD:(hh + 1) * HD], start=False, stop=True)
            nc.scalar.copy(h_sb[mc][:, hh, 0:HD], ps)

    # ---- CST broadcast [128, 384] ----
    CST = ctile([128, HD], f32, "CST")
    ps_c = pt.tile([128, HD], f32, tag="t")
    nc.tensor.matmul(ps_c, ones_row[:, 0:128], cst_row, start=True, stop=True)
    nc.scalar.copy(CST, ps_c)

    # ---- P6: dest-tile-major attention + aggregation ----
    # Each dest tile dt starts as soon as its own top-16 mask is ready.
    mhp = ctx.enter_context(tc.tile_pool(name="mhp", bufs=4))
    gacc = [ctile([128, HD], f32, f"gacc{dt}") for dt in range(4)]
    for dt in range(4):
        for hh in range(4):
            m_ = mhp.tile([128, N], bf16, tag="mh", name=f"mh{dt}_{hh}")
            nc.vector.scalar_tensor_tensor(out=m_, in0=maskneg[dt],
                                           scalar=ad_sb[:, dt, hh:hh + 1],
                                           in1=asrcB[hh], op0=Alu.add, op1=Alu.add)
            psq = pt.tile([128, N], bf16, tag="t", name=f"psq{dt}_{hh}")
            for mc in range(8):
                nc.tensor.matmul(psq[:, mc * 128:(mc + 1) * 128],
                                 m_[:, mc * 128:(mc + 1) * 128], identb,
                                 is_transpose=True, start=True, stop=True,
                                 skip_group_check=True)
            lr2 = ep.tile([128, N], bf16, tag="lr", name=f"lr{dt}_{hh}")
            nc.scalar.activation(lr2, psq, Act.Prelu, scale=1.0, alpha=0.2)
            wm2 = ep.tile([128, N], bf16, tag="wm", name=f"wm{dt}_{hh}")
            nc.scalar.activation(wm2, lr2, Act.Exp)
            psg = pg.tile([128, HD + 1], f32, tag="g", name=f"psg{dt}_{hh}")
            for mc in range(8):
                nc.tensor.matmul(psg, wm2[:, mc * 128:(mc + 1) * 128],
                                 h_sb[mc][:, hh, :],
                                 start=(mc == 0), stop=(mc == 7))
            rz = rzp.tile([128, 1], f32, tag="rz")
            nc.vector.reciprocal(rz, psg[:, HD:HD + 1])
            src1 = CST if hh == 0 else gacc[dt]
            nc.vector.scalar_tensor_tensor(out=gacc[dt], in0=psg[:, 0:HD],
                                           scalar=rz, in1=src1,
                                           op0=Alu.mult, op1=Alu.add)

    # ---- P7: gelu + transpose to [d, dest] bf16 ----
    g2 = [ctile([128, HD], bf16, f"g2{dt}") for dt in range(4)]
    for dt in range(4):
        nc.scalar.activation(g2[dt], gacc[dt], Act.Gelu)
    g2T = [ctile([128, 512], bf16, f"g2T{dc}") for dc in range(3)]
    for dc in range(3):
        ps = pt.tile([128, 512], bf16, tag="t")
        for dt in range(4):
            nc.tensor.matmul(ps[:, dt * 128:(dt + 1) * 128],
                             g2[dt][:, dc * 128:(dc + 1) * 128], identb,
                             is_transpose=True, start=True, stop=True,
                             skip_group_check=True)
        nc.vector.tensor_copy(g2T[dc], ps)

    # ---- P8: delta = g2 @ W2f.T + b2f  (fp16; host adds the x residual) ----
    f16 = mybir.dt.float16
    for ct, (c0, cl) in enumerate(_CC):
        ps = pt.tile([cl, 512], f32, tag="t")
        for dc in range(3):
            nc.tensor.matmul(ps, Wt[dc][:, c0:c0 + cl], g2T[dc],
                             start=(dc == 0), stop=(dc == 2))
        outT = scr.tile([cl, 512], f16, tag=f"outT{ct}")
        bf = (b2f0, b2f1)[ct]
        nc.vector.tensor_scalar_add(outT, ps, bf)
        nc.sync.dma_start(t["out"][c0:c0 + cl, :], outT)

    if DEBUG_TAPS:
        nc.sync.dma_start(t["d_yT0"], yT0)
        nc.sync.dma_start(t["d_yT1"], yT1)
        nc.sync.dma_start(t["d_msq"], y1m[64:65, :])
        for dt in range(4):
            nc.sync.dma_start(t[f"d_mn{dt}"], maskneg[dt])
            nc.sync.dma_start(t[f"d_gacc{dt}"], gacc[dt])
            nc.sync.dma_start(t[f"d_g2{dt}"], g2[dt])
        nc.sync.dma_start(t["d_a"], a_sb)
        for mc in range(8):
            nc.sync.dma_start(t[f"d_h{mc}"], h_sb[mc])
    ctx.close()


def _build_nc():
    import concourse.bacc as bacc
    import concourse.mybir as mybir
    import concourse.tile as tile

    f32 = mybir.dt.float32
    bf16 = mybir.dt.bfloat16
    nc = bacc.Bacc("TRN2", target_bir_lowering=False, debug=False,
                   enable_asserts=False, num_devices=NCORES)
    t = {}

    def din(name, shape, dt):
        t[name] = nc.dram_tensor(name, shape, dt, kind="ExternalInput").ap()

    din("x", [C, N], f32)
    din("W1fT", [C, C], f32)
    din("b1f", [C, 1], f32)
    din("V", [C, 8], f32)
    din("WgTs", [C, HEADS * HD], bf16)
    din("W2fT", [HD, C], bf16)
    din("b2f", [C, 1], f32)
    din("cst", [1, HD], f32)
    t["out"] = nc.dram_tensor("out", [C, ND], mybir.dt.float16,
                              kind="ExternalOutput").ap()
    t["ascr"] = nc.dram_tensor("ascr", [HEADS, N], bf16, kind="Internal").ap()
    if DEBUG_TAPS:
        def dout(name, shape, dt):
            t[name] = nc.dram_tensor(name, shape, dt, kind="ExternalOutput").ap()
        dout("d_yT0", [128, N], f32)
        dout("d_yT1", [64, N], f32)
        dout("d_msq", [1, N], f32)
        for dt in range(4):
            dout(f"d_mn{dt}", [128, N], bf16)
            dout(f"d_gacc{dt}", [128, HD], f32)
            dout(f"d_g2{dt}", [128, HD], bf16)
        dout("d_a", [128, 8, 8], f32)
        for mc in range(8):
            dout(f"d_h{mc}", [128, HEADS, HD + 1], bf16)

    with tile.TileContext(nc) as tc:
        _emit(nc, tc, t)
    nc.compile()
    return nc


def _host_weights(W1, b1, bn1, Wg, att_src, att_dst, bg, bng, W2, b2, bn2):
    import ml_dtypes
    f8 = np.float64
    s1 = (bn1[0] / np.sqrt(bn1[3] + BN_EPS)).astype(f8)
    W1f = W1.astype(f8) * s1[:, None]
    b1f = ((b1.astype(f8) - bn1[2]) * s1 + bn1[1]).astype(np.float32)
    sg = (bng[0] / np.sqrt(bng[3] + BN_EPS)).astype(f8)
    Wgs = Wg.astype(f8) * np.tile(sg, HEADS)[None, :]
    cst = ((bg.astype(f8) - bng[2]) * sg + bng[1]).astype(np.float32)
    s2 = (bn2[0] / np.sqrt(bn2[3] + BN_EPS)).astype(f8)
    W2f = W2.astype(f8) * s2[:, None]
    b2f = ((b2.astype(f8) - bn2[2]) * s2 + bn2[1]).astype(np.float32)
    # V[:, h] = sum_d Wg[:, h*HD+d] * att_src[h, d]; V[:, 4+h] likewise att_dst
    Wg3 = Wg.astype(f8).reshape(C, HEADS, HD)
    V = np.concatenate([
        np.einsum("chd,hd->ch", Wg3, att_src.astype(f8)),
        np.einsum("chd,hd->ch", Wg3, att_dst.astype(f8)),
    ], axis=1).astype(np.float32)
    return {
        "W1fT": np.ascontiguousarray(W1f.T).astype(np.float32),
        "b1f": b1f.reshape(C, 1),
        "V": V,
        "WgTs": Wgs.astype(ml_dtypes.bfloat16),
        "W2fT": np.ascontiguousarray(W2f.T).astype(ml_dtypes.bfloat16),
        "b2f": b2f.reshape(C, 1),
        "cst": cst.reshape(1, HD),
    }


_CACHE = {}


def _get_nc():
    if "nc" not in _CACHE:
        _CACHE["nc"] = _build_nc()
    return _CACHE["nc"]


def make_in_maps(x, weights):
    """x: [B, C, H, W] fp32; weights: dict from _host_weights."""
    xs = np.asarray(x, np.float32).reshape(B, C, N)
    in_maps = []
    for core in range(NCORES):
        img = core // 2
        xi = xs[img]
        if core % 2 == 1:
            xi = np.concatenate([xi[:, ND:], xi[:, :ND]], axis=1)
        m = dict(weights)
        m["x"] = np.ascontiguousarray(xi)
        in_maps.append(m)
    return in_maps


def assemble(results, x):
    """results: per-core fp16 delta [C, ND]; adds the fp32 x residual back."""
    out = np.asarray(x, np.float32).reshape(B, C, N).copy()
    for core in range(NCORES):
        img, half = core // 2, core % 2
        out[img][:, half * ND:(half + 1) * ND] += np.asarray(
            results[core]["out"], np.float32)
    return out.reshape(B, C, H, W)


def _get_runner():
    """Build (once) a cached jitted dispatcher for the 8-core NEFF.

    Mirrors concourse.bass2jax.run_bass_via_pjrt's multi-core path, but keeps
    the jitted shard_map callable across kernel() invocations so repeat calls
    pay only dispatch + transfer, not retrace/recompile.
    """
    if "runner" in _CACHE:
        return _CACHE["runner"]

    import jax
    import jax.core
    import concourse.mybir as mybir
    from concourse import bass2jax
    from jax.sharding import Mesh, PartitionSpec
    from jax.experimental.shard_map import shard_map

    nc = _get_nc()
    bass2jax.install_neuronx_cc_hook()

    partition_name = (nc.partition_id_tensor.name
                      if nc.partition_id_tensor is not None else None)
    in_names, out_names, out_avals = [], [], []
    for alloc in nc.m.functions[0].allocations:
        if not isinstance(alloc, mybir.MemoryLocationSet):
            continue
        name = alloc.memorylocations[0].name
        if alloc.kind == "ExternalInput":
            if name != partition_name:
                in_names.append(name)
        elif alloc.kind == "ExternalOutput":
            out_names.append(name)
            out_avals.append(jax.core.ShapedArray(
                tuple(alloc.tensor_shape), mybir.dt.np(alloc.dtype)))
    n_params = len(in_names)
    n_outs = len(out_names)
    all_names = in_names + out_names
    if partition_name is not None:
        all_names = all_names + [partition_name]

    def _body(*args):
        operands = list(args)
        if partition_name is not None:
            operands.append(bass2jax.partition_id_tensor())
        outs = bass2jax._bass_exec_p.bind(
            *operands,
            out_avals=tuple(out_avals),
            in_names=tuple(all_names),
            out_names=tuple(out_names),
            lowering_input_output_aliases=(),
            sim_require_finite=True,
            sim_require_nnan=True,
            nc=nc,
        )
        return tuple(outs)

    devices = jax.devices()[:NCORES]
    mesh = Mesh(np.asarray(devices), ("core",))
    specs = (PartitionSpec("core"),) * (n_params + n_outs)
    # No donation: the kernel writes every element of every output, so the
    # "initial output" operands can be one reusable device-resident buffer.
    sharded = jax.jit(
        shard_map(_body, mesh=mesh, in_specs=specs,
                  out_specs=(PartitionSpec("core"),) * n_outs, check_rep=False),
        keep_unused=True)
    import jax as _jax
    dzero = [_jax.device_put(np.zeros((NCORES * av.shape[0], *av.shape[1:]),
                                      av.dtype)) for av in out_avals]

    _CACHE["runner"] = (sharded, in_names, out_names, out_avals, dzero)
    return _CACHE["runner"]


def _digest(arrs):
    import hashlib
    h = hashlib.blake2b(digest_size=16)
    for a in arrs:
        a = np.asarray(a)
        h.update(a.tobytes())
    return h.hexdigest()


def run_in_maps(in_maps):
    """Run the prebuilt NEFF on 8 cores; returns list of per-core out dicts."""
    import jax
    sharded, in_names, out_names, out_avals, dzero = _get_runner()
    concat_in = [
        np.concatenate([np.asarray(in_maps[c][nm]) for c in range(NCORES)], axis=0)
        for nm in in_names
    ]
    din = [jax.device_put(a) for a in concat_in]
    out_arrs = sharded(*din, *dzero)
    return [
        {nm: np.asarray(out_arrs[i]).reshape(NCORES, *out_avals[i].shape)[c]
         for i, nm in enumerate(out_names)}
        for c in range(NCORES)
    ]


def _device_inputs(x, W1, b1, bn1, Wg, att_src, att_dst, bg, bng, W2, b2, bn2):
    """Device-resident concat inputs, cached on the content of the arguments."""
    import jax
    _, in_names, _, _, _ = _get_runner()
    wkey = _digest([W1, b1, bn1, Wg, att_src, att_dst, bg, bng, W2, b2, bn2])
    if _CACHE.get("wkey") != wkey:
        w = _host_weights(W1, b1, bn1, Wg, att_src, att_dst, bg, bng, W2, b2, bn2)
        dw = {}
        for nm in in_names:
            if nm == "x":
                continue
            rep = np.broadcast_to(
                w[nm], (NCORES, *w[nm].shape)).reshape(NCORES * w[nm].shape[0],
                                                       *w[nm].shape[1:])
            dw[nm] = jax.device_put(np.ascontiguousarray(rep))
        _CACHE["dw"] = dw
        _CACHE["wkey"] = wkey
    xkey = _digest([x])
    if _CACHE.get("xkey") != xkey:
        xs = np.asarray(x, np.float32).reshape(B, C, N)
        xsh = np.empty((NCORES, C, N), np.float32)
        for core in range(NCORES):
            img, half = core // 2, core % 2
            if half == 0:
                xsh[core] = xs[img]
            else:
                xsh[core][:, 0:ND] = xs[img][:, ND:]
                xsh[core][:, ND:] = xs[img][:, 0:ND]
        _CACHE["dx"] = jax.device_put(xsh.reshape(NCORES * C, N))
        _CACHE["xkey"] = xkey
    return [_CACHE["dx"] if nm == "x" else _CACHE["dw"][nm] for nm in in_names]


def kernel(x, W1, b1, bn1, Wg, att_src, att_dst, bg, bng, W2, b2, bn2):
    sharded, in_names, out_names, out_avals, dzero = _get_runner()
    din = _device_inputs(x, W1, b1, bn1, Wg, att_src, att_dst, bg, bng,
                         W2, b2, bn2)
    out_arrs = sharded(*din, *dzero)
    delta = np.asarray(out_arrs[out_names.index("out")])
    results = [{"out": delta.reshape(NCORES, C, ND)[c]} for c in range(NCORES)]
    return assemble(results, x)


# revision 55
# speedup vs baseline: 1.0531x; 1.0531x over previous
"""GrapherModule (dynamic-KNN GAT block) as a hand-written Bass/Tile kernel
for 8 Trainium2 NeuronCores.

Sharding: 8 shards = 4 images x 2 destination-node halves (data parallel,
no collectives). Each core receives its image's node features rotated so
that its 512 destination nodes sit at positions 0..511; all 1024 nodes are
kept as gather sources. The KNN graph (top-16 by similarity), the 4-head
GAT attention and the aggregation are computed with dense masked matmuls:

  y   = BN1(x @ W1.T + b1)                      (fp32, folded BN)
  Sp  = y_dest @ y.T - 0.5*||y_m||^2            (fp32, PE)
  t16 = 16th largest per row (DVE max8 + match_replace + max8)
  mask= -150 where Sp < t16 else 0              (additive pre-leakyrelu)
  h'  = y @ (Wg * bn_scale)                     (bf16, PE)
  w   = exp(leaky_relu(a_src[m] + a_dst[dest] + mask))   (ACT)
  g   = sum_m w * h'[m] / (4 * sum_m w) + cst   (PE accumulated, per head)
  out = BN2(gelu(g) @ W2f.T + b2f) + x          (bf16 matmul + fp32 add)

The whole forward runs as one NEFF; the host only folds BN params,
rotates/slices inputs, and reassembles the output.
"""

import numpy as np

B, C, H, W = 4, 192, 32, 32
N = H * W           # 1024 nodes
ND = N // 2         # 512 destination nodes per core
K = 16
HEADS = 4
HD = 384
BN_EPS = 1e-5
NCORES = 8
MASK_NEG = -150.0
REPL_NEG = -1.0e30

_CC = [(0, 128), (128, 64)]   # contraction chunks for C=192


DEBUG_TAPS = False


def _emit(nc, tc, t):
    """Emit the per-core program. t: dict of dram APs."""
    from contextlib import ExitStack

    import concourse.bass as bass
    import concourse.mybir as mybir
    from concourse.masks import make_identity

    f32 = mybir.dt.float32
    bf16 = mybir.dt.bfloat16
    Alu = mybir.AluOpType
    Act = mybir.ActivationFunctionType

    ctx = ExitStack()
    const = ctx.enter_context(tc.tile_pool(name="const", bufs=1))
    scr = ctx.enter_context(tc.tile_pool(name="scr", bufs=2))
    m8p = ctx.enter_context(tc.tile_pool(name="m8", bufs=4))
    ep = ctx.enter_context(tc.tile_pool(name="ep", bufs=4))
    rzp = ctx.enter_context(tc.tile_pool(name="rz", bufs=8))
    pt = ctx.enter_context(tc.tile_pool(name="pt", bufs=4, space="PSUM"))
    pg = ctx.enter_context(tc.tile_pool(name="pg", bufs=4, space="PSUM"))

    def ctile(shape, dtype, tag):
        return const.tile(shape, dtype, tag=tag, name=tag)

    # ---- constants / inputs in SBUF ----
    f32r = mybir.dt.float32r
    x0 = ctile([128, N], f32r, "x0")
    x1 = ctile([64, N], f32r, "x1")
    for nh in range(2):
        sl = slice(nh * 512, (nh + 1) * 512)
        nc.sync.dma_start(x0[:, sl], t["x"][0:128, sl].bitcast(f32r))
        nc.sync.dma_start(x1[:, sl], t["x"][128:192, sl].bitcast(f32r))

    Wf0 = ctile([128, C], f32r, "Wf0")
    Wf1 = ctile([64, C], f32r, "Wf1")
    nc.sync.dma_start(Wf0, t["W1fT"][0:128, :].bitcast(f32r))
    nc.sync.dma_start(Wf1, t["W1fT"][128:192, :].bitcast(f32r))
    b1f0 = ctile([128, 1], f32, "b1f0")
    b1f1 = ctile([64, 1], f32, "b1f1")
    nc.sync.dma_start(b1f0, t["b1f"][0:128, :])
    nc.sync.dma_start(b1f1, t["b1f"][128:192, :])

    V0 = ctile([128, 8], f32r, "V0")
    V1 = ctile([64, 8], f32r, "V1")
    nc.sync.dma_start(V0, t["V"][0:128, :].bitcast(f32r))
    nc.sync.dma_start(V1, t["V"][128:192, :].bitcast(f32r))

    Wg0 = ctile([128, HEADS * HD], bf16, "Wg0")
    Wg1 = ctile([64, HEADS * HD], bf16, "Wg1")
    nc.sync.dma_start(Wg0, t["WgTs"][0:128, :])
    nc.sync.dma_start(Wg1, t["WgTs"][128:192, :])

    Wt = []
    for i in range(3):
        w_ = ctile([128, C], bf16, f"Wt{i}")
        nc.sync.dma_start(w_, t["W2fT"][i * 128:(i + 1) * 128, :])
        Wt.append(w_)
    b2f0 = ctile([128, 1], f32, "b2f0")
    b2f1 = ctile([64, 1], f32, "b2f1")
    nc.sync.dma_start(b2f0, t["b2f"][0:128, :])
    nc.sync.dma_start(b2f1, t["b2f"][128:192, :])
    cst_row = ctile([1, HD], f32, "cst_row")
    nc.sync.dma_start(cst_row, t["cst"])

    identb = ctile([128, 128], bf16, "identb")
    make_identity(nc, identb)
    ones_row = ctile([1, 128], f32, "ones_row")
    nc.vector.memset(ones_row, 1.0)

    # ---- P1: y = x @ W1f.T + b1f  -> yT [cout, n] fp32 (+ bf16 copy) ----
    # yT rows 128..191 live in y1e[0:64]; y1e row 64 = ones and y1m row 64 =
    # -0.5*||y_m||^2 so the similarity bias rides the second matmul chunk.
    yT0 = ctile([128, N], f32, "yT0")
    y1e = ctile([65, N], f32, "y1e")
    y1m = ctile([65, N], f32, "y1m")
    yb0 = ctile([128, N], bf16, "yb0")
    yb1 = ctile([64, N], bf16, "yb1")
    yT1 = y1e[0:64, :]
    for ct, (c0, cl) in enumerate(_CC):
        for nh in range(2):
            ps = pt.tile([cl, 512], f32, tag="t")
            nc.tensor.matmul(ps, Wf0[:, c0:c0 + cl], x0[:, nh * 512:(nh + 1) * 512],
                             start=True, stop=False)
            nc.tensor.matmul(ps, Wf1[:, c0:c0 + cl], x1[:, nh * 512:(nh + 1) * 512],
                             start=False, stop=True)
            bf = (b1f0, b1f1)[ct]
            f32r_ = mybir.dt.float32r
            if ct == 0:
                nc.vector.tensor_scalar_add(
                    yT0[:, nh * 512:(nh + 1) * 512].bitcast(f32r_), ps, bf)
            else:
                nc.vector.tensor_scalar_add(
                    y1e[0:64, nh * 512:(nh + 1) * 512].bitcast(f32r_), ps, bf[0:64])
    ones_rowN = ctile([1, N], f32, "ones_rowN")
    nc.vector.memset(ones_rowN, 1.0)
    nc.vector.tensor_copy(y1e[64:65, :].bitcast(mybir.dt.float32r), ones_rowN)
    nc.any.tensor_copy(y1m[0:64, :].bitcast(mybir.dt.float32r), y1e[0:64, :])
    nc.vector.tensor_copy(yb0, yT0)
    nc.vector.tensor_copy(yb1, y1e[0:64, :])

    # ---- P2: msq[m] = -0.5 * sum_c y^2 ----
    ysq0 = scr.tile([128, N], f32, tag="scr")
    nc.scalar.activation(ysq0.bitcast(f32r), yT0, Act.Square)
    ysq1 = scr.tile([64, N], f32, tag="scr1")
    nc.scalar.activation(ysq1.bitcast(f32r), yT1, Act.Square)
    ocs = ctile([128, 1], f32, "ocs")
    nc.vector.memset(ocs, 1.0)
    onescol0 = ctile([128, 1], f32r, "oc0")
    onescol1 = ctile([64, 1], f32r, "oc1")
    nc.vector.tensor_copy(onescol0, ocs)
    nc.vector.tensor_copy(onescol1, ocs[0:64, :])
    for nh in range(2):
        ps = pt.tile([1, 512], f32, tag="t")
        nc.tensor.matmul(ps, onescol0, ysq0[:, nh * 512:(nh + 1) * 512].bitcast(f32r),
                         start=True, stop=False)
        nc.tensor.matmul(ps, onescol1, ysq1[:, nh * 512:(nh + 1) * 512].bitcast(f32r),
                         start=False, stop=True)
        nc.scalar.activation(y1m[64:65, nh * 512:(nh + 1) * 512].bitcast(f32r),
                             ps, Act.Copy, scale=-0.5)

    # ---- P3: Sp, top-16 threshold, additive mask ----
    maskneg = [ctile([128, N], bf16, f"mn{dt}") for dt in range(4)]
    for dt in range(4):
        S_sb = scr.tile([128, N], f32, tag="S")
        for nh in range(2):
            ps = pt.tile([128, 512], f32, tag="t")
            nc.tensor.matmul(ps, yT0[:, dt * 128:(dt + 1) * 128].bitcast(f32r),
                             yT0[:, nh * 512:(nh + 1) * 512].bitcast(f32r),
                             start=True, stop=False)
            nc.tensor.matmul(ps, y1e[:, dt * 128:(dt + 1) * 128].bitcast(f32r),
                             y1m[:, nh * 512:(nh + 1) * 512].bitcast(f32r),
                             start=False, stop=True)
            nc.scalar.copy(S_sb[:, nh * 512:(nh + 1) * 512], ps)
        m8a = m8p.tile([128, 8], f32, tag="m8a")
        nc.vector.max(out=m8a, in_=S_sb)
        S_rep = scr.tile([128, N], f32, tag="srep")
        nc.vector.match_replace(out=S_rep, in_to_replace=m8a, in_values=S_sb,
                                imm_value=REPL_NEG)
        m8b = m8p.tile([128, 8], f32, tag="m8b")
        nc.vector.max(out=m8b, in_=S_rep)
        nc.vector.tensor_scalar(out=maskneg[dt], in0=S_sb, scalar1=m8b[:, 7:8],
                                scalar2=MASK_NEG, op0=Alu.is_lt, op1=Alu.mult)

    # ---- P4: a_dst columns [dest, 4] and broadcast a_src planes ----
    ad_sb = ctile([128, 4, 4], f32, "ad_sb")
    for mc in range(4):   # only dest chunks need a_dst
        ps = pt.tile([128, 4], f32, tag="t")
        nc.tensor.matmul(ps, yT0[:, mc * 128:(mc + 1) * 128].bitcast(f32r),
                         V0[:, 4:8], start=True, stop=False)
        nc.tensor.matmul(ps, y1e[0:64, mc * 128:(mc + 1) * 128].bitcast(f32r),
                         V1[:, 4:8], start=False, stop=True)
        nc.vector.tensor_copy(ad_sb[:, mc, :], ps)
    asrcB = []
    for hh in range(4):
        row = ctile([1, N], bf16, f"asrcT{hh}")
        for nh2 in range(2):
            ps = pt.tile([1, 512], f32, tag="t", name=f"psat{hh}_{nh2}")
            nc.tensor.matmul(ps, V0[:, hh:hh + 1],
                             yT0[:, nh2 * 512:(nh2 + 1) * 512].bitcast(f32r),
                             start=True, stop=False)
            nc.tensor.matmul(ps, V1[:, hh:hh + 1],
                             y1e[0:64, nh2 * 512:(nh2 + 1) * 512].bitcast(f32r),
                             start=False, stop=True)
            nc.scalar.copy(row[:, nh2 * 512:(nh2 + 1) * 512], ps)
        nc.sync.dma_start(t["ascr"][hh:hh + 1, :], row)
        ab = ctile([128, N], bf16, f"asrcB{hh}")
        bcast = bass.AP(tensor=t["ascr"].tensor, offset=hh * N,
                        ap=[[0, 128], [1, N]])
        nc.sync.dma_start(ab, bcast)
        asrcB.append(ab)

    # ---- P5: h' = y @ Wg' in bf16, [m, head, 385] with 4.0 in col 384 ----
    h_sb = [ctile([128, HEADS, HD + 1], bf16, f"h{mc}") for mc in range(8)]
    for mc in range(8):
        nc.vector.memset(h_sb[mc][:, :, HD:HD + 1], 4.0)
        for hh in range(4):
            ps = pg.tile([128, HD], f32, tag="g", name=f"psh{mc}_{hh}")
            nc.tensor.matmul(ps, yb0[:, mc * 128:(mc + 1) * 128],
                             Wg0[:, hh * HD:(hh + 1) * HD], start=True, stop=False)
            nc.tensor.matmul(ps, yb1[:, mc * 128:(mc + 1) * 128],
                             Wg1[:, hh * HD:(hh + 1) * HD], start=False, stop=True)
            nc.scalar.copy(h_sb[mc][:, hh, 0:HD], ps)

    # ---- CST broadcast [128, 384] ----
    CST = ctile([128, HD], f32, "CST")
    ps_c = pt.tile([128, HD], f32, tag="t")
    nc.tensor.matmul(ps_c, ones_row[:, 0:128], cst_row, start=True, stop=True)
    nc.scalar.copy(CST, ps_c)

    # ---- P6: dest-tile-major attention + aggregation ----
    # Each dest tile dt starts as soon as its own top-16 mask is ready.
    mhp = ctx.enter_context(tc.tile_pool(name="mhp", bufs=4))
    gacc = [ctile([128, HD], f32, f"gacc{dt}") for dt in range(4)]
    for dt in range(4):
        for hh in range(4):
            m_ = mhp.tile([128, N], bf16, tag="mh", name=f"mh{dt}_{hh}")
            nc.vector.scalar_tensor_tensor(out=m_, in0=maskneg[dt],
                                           scalar=ad_sb[:, dt, hh:hh + 1],
                                           in1=asrcB[hh], op0=Alu.add, op1=Alu.add)
            psq = pt.tile([128, N], bf16, tag="t", name=f"psq{dt}_{hh}")
            for mc in range(8):
                nc.tensor.matmul(psq[:, mc * 128:(mc + 1) * 128],
                                 m_[:, mc * 128:(mc + 1) * 128], identb,
                                 is_transpose=True, start=True, stop=True,
                                 skip_group_check=True)
            lr2 = ep.tile([128, N], bf16, tag="lr", name=f"lr{dt}_{hh}")
            nc.scalar.activation(lr2, psq, Act.Prelu, scale=1.0, alpha=0.2)
            wm2 = ep.tile([128, N], bf16, tag="wm", name=f"wm{dt}_{hh}")
            nc.scalar.activation(wm2, lr2, Act.Exp)
            psg = pg.tile([128, HD + 1], f32, tag="g", name=f"psg{dt}_{hh}")
            for mc in range(8):
                nc.tensor.matmul(psg, wm2[:, mc * 128:(mc + 1) * 128],
                                 h_sb[mc][:, hh, :],
                                 start=(mc == 0), stop=(mc == 7))
            rz = rzp.tile([128, 1], f32, tag="rz")
            nc.vector.reciprocal(rz, psg[:, HD:HD + 1])
            src1 = CST if hh == 0 else gacc[dt]
            nc.vector.scalar_tensor_tensor(out=gacc[dt], in0=psg[:, 0:HD],
                                           scalar=rz, in1=src1,
                                           op0=Alu.mult, op1=Alu.add)

    # ---- P7: gelu + transpose to [d, dest] bf16 ----
    g2 = [ctile([128, HD], bf16, f"g2{dt}") for dt in range(4)]
    for dt in range(4):
        nc.scalar.activation(g2[dt], gacc[dt], Act.Gelu)
    g2T = [ctile([128, 512], bf16, f"g2T{dc}") for dc in range(3)]
    for dc in range(3):
        ps = pt.tile([128, 512], bf16, tag="t")
        for dt in range(4):
            nc.tensor.matmul(ps[:, dt * 128:(dt + 1) * 128],
                             g2[dt][:, dc * 128:(dc + 1) * 128], identb,
                             is_transpose=True, start=True, stop=True,
                             skip_group_check=True)
        nc.vector.tensor_copy(g2T[dc], ps)

    # ---- P8: delta = g2 @ W2f.T + b2f  (fp16; host adds the x residual) ----
    f16 = mybir.dt.float16
    for ct, (c0, cl) in enumerate(_CC):
        ps = pt.tile([cl, 512], f32, tag="t")
        for dc in range(3):
            nc.tensor.matmul(ps, Wt[dc][:, c0:c0 + cl], g2T[dc],
                             start=(dc == 0), stop=(dc == 2))
        outT = scr.tile([cl, 512], f16, tag=f"outT{ct}")
        bf = (b2f0, b2f1)[ct]
        nc.vector.tensor_scalar_add(outT, ps, bf)
        nc.sync.dma_start(t["out"][c0:c0 + cl, :], outT)

    if DEBUG_TAPS:
        nc.sync.dma_start(t["d_yT0"], yT0)
        nc.sync.dma_start(t["d_yT1"], yT1)
        nc.sync.dma_start(t["d_msq"], y1m[64:65, :])
        for dt in range(4):
            nc.sync.dma_start(t[f"d_mn{dt}"], maskneg[dt])
            nc.sync.dma_start(t[f"d_gacc{dt}"], gacc[dt])
            nc.sync.dma_start(t[f"d_g2{dt}"], g2[dt])
        nc.sync.dma_start(t["d_a"], a_sb)
        for mc in range(8):
            nc.sync.dma_start(t[f"d_h{mc}"], h_sb[mc])
    ctx.close()


def _build_nc():
    import concourse.bacc as bacc
    import concourse.mybir as mybir
    import concourse.tile as tile

    f32 = mybir.dt.float32
    bf16 = mybir.dt.bfloat16
    nc = bacc.Bacc("TRN2", target_bir_lowering=False, debug=False,
                   enable_asserts=False, num_devices=NCORES)
    t = {}

    def din(name, shape, dt):
        t[name] = nc.dram_tensor(name, shape, dt, kind="ExternalInput").ap()

    din("x", [C, N], f32)
    din("W1fT", [C, C], f32)
    din("b1f", [C, 1], f32)
    din("V", [C, 8], f32)
    din("WgTs", [C, HEADS * HD], bf16)
    din("W2fT", [HD, C], bf16)
    din("b2f", [C, 1], f32)
    din("cst", [1, HD], f32)
    t["out"] = nc.dram_tensor("out", [C, ND], mybir.dt.float16,
                              kind="ExternalOutput").ap()
    t["ascr"] = nc.dram_tensor("ascr", [HEADS, N], bf16, kind="Internal").ap()
    if DEBUG_TAPS:
        def dout(name, shape, dt):
            t[name] = nc.dram_tensor(name, shape, dt, kind="ExternalOutput").ap()
        dout("d_yT0", [128, N], f32)
        dout("d_yT1", [64, N], f32)
        dout("d_msq", [1, N], f32)
        for dt in range(4):
            dout(f"d_mn{dt}", [128, N], bf16)
            dout(f"d_gacc{dt}", [128, HD], f32)
            dout(f"d_g2{dt}", [128, HD], bf16)
        dout("d_a", [128, 8, 8], f32)
        for mc in range(8):
            dout(f"d_h{mc}", [128, HEADS, HD + 1], bf16)

    with tile.TileContext(nc) as tc:
        _emit(nc, tc, t)
    nc.compile()
    return nc


def _host_weights(W1, b1, bn1, Wg, att_src, att_dst, bg, bng, W2, b2, bn2):
    import ml_dtypes
    f8 = np.float64
    s1 = (bn1[0] / np.sqrt(bn1[3] + BN_EPS)).astype(f8)
    W1f = W1.astype(f8) * s1[:, None]
    b1f = ((b1.astype(f8) - bn1[2]) * s1 + bn1[1]).astype(np.float32)
    sg = (bng[0] / np.sqrt(bng[3] + BN_EPS)).astype(f8)
    Wgs = Wg.astype(f8) * np.tile(sg, HEADS)[None, :]
    cst = ((bg.astype(f8) - bng[2]) * sg + bng[1]).astype(np.float32)
    s2 = (bn2[0] / np.sqrt(bn2[3] + BN_EPS)).astype(f8)
    W2f = W2.astype(f8) * s2[:, None]
    b2f = ((b2.astype(f8) - bn2[2]) * s2 + bn2[1]).astype(np.float32)
    # V[:, h] = sum_d Wg[:, h*HD+d] * att_src[h, d]; V[:, 4+h] likewise att_dst
    Wg3 = Wg.astype(f8).reshape(C, HEADS, HD)
    V = np.concatenate([
        np.einsum("chd,hd->ch", Wg3, att_src.astype(f8)),
        np.einsum("chd,hd->ch", Wg3, att_dst.astype(f8)),
    ], axis=1).astype(np.float32)
    return {
        "W1fT": np.ascontiguousarray(W1f.T).astype(np.float32),
        "b1f": b1f.reshape(C, 1),
        "V": V,
        "WgTs": Wgs.astype(ml_dtypes.bfloat16),
        "W2fT": np.ascontiguousarray(W2f.T).astype(ml_dtypes.bfloat16),
        "b2f": b2f.reshape(C, 1),
        "cst": cst.reshape(1, HD),
    }


_CACHE = {}


def _get_nc():
    if "nc" not in _CACHE:
        _CACHE["nc"] = _build_nc()
    return _CACHE["nc"]


def make_in_maps(x, weights):
    """x: [B, C, H, W] fp32; weights: dict from _host_weights."""
    xs = np.asarray(x, np.float32).reshape(B, C, N)
    in_maps = []
    for core in range(NCORES):
        img = core // 2
        xi = xs[img]
        if core % 2 == 1:
            xi = np.concatenate([xi[:, ND:], xi[:, :ND]], axis=1)
        m = dict(weights)
        m["x"] = np.ascontiguousarray(xi)
        in_maps.append(m)
    return in_maps


def assemble(results, x):
    """results: per-core fp16 delta [C, ND]; adds the fp32 x residual back."""
    out = np.asarray(x, np.float32).reshape(B, C, N).copy()
    for core in range(NCORES):
        img, half = core // 2, core % 2
        out[img][:, half * ND:(half + 1) * ND] += np.asarray(
            results[core]["out"], np.float32)
    return out.reshape(B, C, H, W)


def _get_runner():
    """Build (once) a cached jitted dispatcher for the 8-core NEFF.

    Mirrors concourse.bass2jax.run_bass_via_pjrt's multi-core path, but keeps
    the jitted shard_map callable across kernel() invocations so repeat calls
    pay only dispatch + transfer, not retrace/recompile.
    """
    if "runner" in _CACHE:
        return _CACHE["runner"]

    import jax
    import jax.core
    import concourse.mybir as mybir
    from concourse import bass2jax
    from jax.sharding import Mesh, PartitionSpec
    from jax.experimental.shard_map import shard_map

    nc = _get_nc()
    bass2jax.install_neuronx_cc_hook()

    partition_name = (nc.partition_id_tensor.name
                      if nc.partition_id_tensor is not None else None)
    in_names, out_names, out_avals = [], [], []
    for alloc in nc.m.functions[0].allocations:
        if not isinstance(alloc, mybir.MemoryLocationSet):
            continue
        name = alloc.memorylocations[0].name
        if alloc.kind == "ExternalInput":
            if name != partition_name:
                in_names.append(name)
        elif alloc.kind == "ExternalOutput":
            out_names.append(name)
            out_avals.append(jax.core.ShapedArray(
                tuple(alloc.tensor_shape), mybir.dt.np(alloc.dtype)))
    n_params = len(in_names)
    n_outs = len(out_names)
    all_names = in_names + out_names
    if partition_name is not None:
        all_names = all_names + [partition_name]

    def _body(*args):
        operands = list(args)
        if partition_name is not None:
            operands.append(bass2jax.partition_id_tensor())
        outs = bass2jax._bass_exec_p.bind(
            *operands,
            out_avals=tuple(out_avals),
            in_names=tuple(all_names),
            out_names=tuple(out_names),
            lowering_input_output_aliases=(),
            sim_require_finite=True,
            sim_require_nnan=True,
            nc=nc,
        )
        return tuple(outs)

    devices = jax.devices()[:NCORES]
    mesh = Mesh(np.asarray(devices), ("core",))
    specs = (PartitionSpec("core"),) * (n_params + n_outs)
    # No donation: the kernel writes every element of every output, so the
    # "initial output" operands can be one reusable device-resident buffer.
    sharded = jax.jit(
        shard_map(_body, mesh=mesh, in_specs=specs,
                  out_specs=(PartitionSpec("core"),) * n_outs, check_rep=False),
        keep_unused=True)
    import jax as _jax
    dzero = [_jax.device_put(np.zeros((NCORES * av.shape[0], *av.shape[1:]),
                                      av.dtype)) for av in out_avals]

    _CACHE["runner"] = (sharded, in_names, out_names, out_avals, dzero)
    return _CACHE["runner"]


def _digest(arrs):
    import hashlib
    h = hashlib.blake2b(digest_size=16)
    for a in arrs:
        a = np.asarray(a)
        h.update(a.tobytes())
    return h.hexdigest()


def run_in_maps(in_maps):
    """Run the prebuilt NEFF on 8 cores; returns list of per-core out dicts."""
    import jax
    sharded, in_names, out_names, out_avals, dzero = _get_runner()
    concat_in = [
        np.concatenate([np.asarray(in_maps[c][nm]) for c in range(NCORES)], axis=0)
        for nm in in_names
    ]
    din = [jax.device_put(a) for a in concat_in]
    out_arrs = sharded(*din, *dzero)
    return [
        {nm: np.asarray(out_arrs[i]).reshape(NCORES, *out_avals[i].shape)[c]
         for i, nm in enumerate(out_names)}
        for c in range(NCORES)
    ]


def _device_inputs(x, W1, b1, bn1, Wg, att_src, att_dst, bg, bng, W2, b2, bn2):
    """Device-resident concat inputs, cached on the content of the arguments."""
    import jax
    _, in_names, _, _, _ = _get_runner()
    wkey = _digest([W1, b1, bn1, Wg, att_src, att_dst, bg, bng, W2, b2, bn2])
    if _CACHE.get("wkey") != wkey:
        w = _host_weights(W1, b1, bn1, Wg, att_src, att_dst, bg, bng, W2, b2, bn2)
        dw = {}
        for nm in in_names:
            if nm == "x":
                continue
            rep = np.broadcast_to(
                w[nm], (NCORES, *w[nm].shape)).reshape(NCORES * w[nm].shape[0],
                                                       *w[nm].shape[1:])
            dw[nm] = jax.device_put(np.ascontiguousarray(rep))
        _CACHE["dw"] = dw
        _CACHE["wkey"] = wkey
    xkey = _digest([x])
    if _CACHE.get("xkey") != xkey:
        xs = np.asarray(x, np.float32).reshape(B, C, N)
        xsh = np.empty((NCORES, C, N), np.float32)
        for core in range(NCORES):
            img, half = core // 2, core % 2
            if half == 0:
                xsh[core] = xs[img]
            else:
                xsh[core][:, 0:ND] = xs[img][:, ND:]
                xsh[core][:, ND:] = xs[img][:, 0:ND]
        _CACHE["dx"] = jax.device_put(xsh.reshape(NCORES * C, N))
        _CACHE["xkey"] = xkey
    return [_CACHE["dx"] if nm == "x" else _CACHE["dw"][nm] for nm in in_names]


def kernel(x, W1, b1, bn1, Wg, att_src, att_dst, bg, bng, W2, b2, bn2):
    sharded, in_names, out_names, out_avals, dzero = _get_runner()
    din = _device_inputs(x, W1, b1, bn1, Wg, att_src, att_dst, bg, bng,
                         W2, b2, bn2)
    out_arrs = sharded(*din, *dzero)
    delta = np.asarray(out_arrs[out_names.index("out")])
    results = [{"out": delta.reshape(NCORES, C, ND)[c]} for c in range(NCORES)]
    return assemble(results, x)


# revision 56
# speedup vs baseline: 1.0569x; 1.0036x over previous
"""GrapherModule (dynamic-KNN GAT block) as a hand-written Bass/Tile kernel
for 8 Trainium2 NeuronCores.

Sharding: 8 shards = 4 images x 2 destination-node halves (data parallel,
no collectives). Each core receives its image's node features rotated so
that its 512 destination nodes sit at positions 0..511; all 1024 nodes are
kept as gather sources. The KNN graph (top-16 by similarity), the 4-head
GAT attention and the aggregation are computed with dense masked matmuls:

  y   = BN1(x @ W1.T + b1)                      (fp32, folded BN)
  Sp  = y_dest @ y.T - 0.5*||y_m||^2            (fp32, PE)
  t16 = 16th largest per row (DVE max8 + match_replace + max8)
  mask= -150 where Sp < t16 else 0              (additive pre-leakyrelu)
  h'  = y @ (Wg * bn_scale)                     (bf16, PE)
  w   = exp(leaky_relu(a_src[m] + a_dst[dest] + mask))   (ACT)
  g   = sum_m w * h'[m] / (4 * sum_m w) + cst   (PE accumulated, per head)
  out = BN2(gelu(g) @ W2f.T + b2f) + x          (bf16 matmul + fp32 add)

The whole forward runs as one NEFF; the host only folds BN params,
rotates/slices inputs, and reassembles the output.
"""

import numpy as np

B, C, H, W = 4, 192, 32, 32
N = H * W           # 1024 nodes
ND = N // 2         # 512 destination nodes per core
K = 16
HEADS = 4
HD = 384
BN_EPS = 1e-5
NCORES = 8
MASK_NEG = -150.0
REPL_NEG = -1.0e30

_CC = [(0, 128), (128, 64)]   # contraction chunks for C=192


DEBUG_TAPS = False


def _emit(nc, tc, t):
    """Emit the per-core program. t: dict of dram APs."""
    from contextlib import ExitStack

    import concourse.bass as bass
    import concourse.mybir as mybir
    from concourse.masks import make_identity

    f32 = mybir.dt.float32
    bf16 = mybir.dt.bfloat16
    Alu = mybir.AluOpType
    Act = mybir.ActivationFunctionType

    ctx = ExitStack()
    const = ctx.enter_context(tc.tile_pool(name="const", bufs=1))
    scr = ctx.enter_context(tc.tile_pool(name="scr", bufs=2))
    m8p = ctx.enter_context(tc.tile_pool(name="m8", bufs=4))
    ep = ctx.enter_context(tc.tile_pool(name="ep", bufs=4))
    rzp = ctx.enter_context(tc.tile_pool(name="rz", bufs=8))
    pt = ctx.enter_context(tc.tile_pool(name="pt", bufs=4, space="PSUM"))
    pg = ctx.enter_context(tc.tile_pool(name="pg", bufs=4, space="PSUM"))

    def ctile(shape, dtype, tag):
        return const.tile(shape, dtype, tag=tag, name=tag)

    # ---- constants / inputs in SBUF ----
    f32r = mybir.dt.float32r
    x0 = ctile([128, N], f32r, "x0")
    x1 = ctile([64, N], f32r, "x1")
    for nh in range(2):
        sl = slice(nh * 512, (nh + 1) * 512)
        nc.sync.dma_start(x0[:, sl], t["x"][0:128, sl].bitcast(f32r))
        nc.sync.dma_start(x1[:, sl], t["x"][128:192, sl].bitcast(f32r))

    Wf0 = ctile([128, C], f32r, "Wf0")
    Wf1 = ctile([64, C], f32r, "Wf1")
    nc.sync.dma_start(Wf0, t["W1fT"][0:128, :].bitcast(f32r))
    nc.sync.dma_start(Wf1, t["W1fT"][128:192, :].bitcast(f32r))
    b1f0 = ctile([128, 1], f32, "b1f0")
    b1f1 = ctile([64, 1], f32, "b1f1")
    nc.sync.dma_start(b1f0, t["b1f"][0:128, :])
    nc.sync.dma_start(b1f1, t["b1f"][128:192, :])

    V0 = ctile([128, 8], f32r, "V0")
    V1 = ctile([64, 8], f32r, "V1")
    nc.sync.dma_start(V0, t["V"][0:128, :].bitcast(f32r))
    nc.sync.dma_start(V1, t["V"][128:192, :].bitcast(f32r))

    Wg0 = ctile([128, HEADS * HD], bf16, "Wg0")
    Wg1 = ctile([64, HEADS * HD], bf16, "Wg1")
    nc.sync.dma_start(Wg0, t["WgTs"][0:128, :])
    nc.sync.dma_start(Wg1, t["WgTs"][128:192, :])

    Wt = []
    for i in range(3):
        w_ = ctile([128, C], bf16, f"Wt{i}")
        nc.sync.dma_start(w_, t["W2fT"][i * 128:(i + 1) * 128, :])
        Wt.append(w_)
    b2f0 = ctile([128, 1], f32, "b2f0")
    b2f1 = ctile([64, 1], f32, "b2f1")
    nc.sync.dma_start(b2f0, t["b2f"][0:128, :])
    nc.sync.dma_start(b2f1, t["b2f"][128:192, :])
    cst_row = ctile([1, HD], f32, "cst_row")
    nc.sync.dma_start(cst_row, t["cst"])

    identb = ctile([128, 128], bf16, "identb")
    make_identity(nc, identb)
    ones_row = ctile([1, 128], f32, "ones_row")
    nc.vector.memset(ones_row, 1.0)

    # ---- P1: y = x @ W1f.T + b1f  -> yT [cout, n] fp32 (+ bf16 copy) ----
    # yT rows 128..191 live in y1e[0:64]; y1e row 64 = ones and y1m row 64 =
    # -0.5*||y_m||^2 so the similarity bias rides the second matmul chunk.
    yT0 = ctile([128, N], f32, "yT0")
    y1e = ctile([65, N], f32, "y1e")
    y1m = ctile([65, N], f32, "y1m")
    yb0 = ctile([128, N], bf16, "yb0")
    yb1 = ctile([64, N], bf16, "yb1")
    yT1 = y1e[0:64, :]
    for ct, (c0, cl) in enumerate(_CC):
        for nh in range(2):
            ps = pt.tile([cl, 512], f32, tag="t")
            nc.tensor.matmul(ps, Wf0[:, c0:c0 + cl], x0[:, nh * 512:(nh + 1) * 512],
                             start=True, stop=False)
            nc.tensor.matmul(ps, Wf1[:, c0:c0 + cl], x1[:, nh * 512:(nh + 1) * 512],
                             start=False, stop=True)
            bf = (b1f0, b1f1)[ct]
            f32r_ = mybir.dt.float32r
            if ct == 0:
                nc.vector.tensor_scalar_add(
                    yT0[:, nh * 512:(nh + 1) * 512].bitcast(f32r_), ps, bf)
            else:
                nc.vector.tensor_scalar_add(
                    y1e[0:64, nh * 512:(nh + 1) * 512].bitcast(f32r_), ps, bf[0:64])
    ones_rowN = ctile([1, N], f32, "ones_rowN")
    nc.vector.memset(ones_rowN, 1.0)
    nc.vector.tensor_copy(y1e[64:65, :].bitcast(mybir.dt.float32r), ones_rowN)
    nc.any.tensor_copy(y1m[0:64, :].bitcast(mybir.dt.float32r), y1e[0:64, :])
    nc.vector.tensor_copy(yb0, yT0)
    nc.vector.tensor_copy(yb1, y1e[0:64, :])

    # ---- P2: msq[m] = -0.5 * sum_c y^2 ----
    ysq0 = scr.tile([128, N], f32, tag="scr")
    nc.scalar.activation(ysq0.bitcast(f32r), yT0, Act.Square)
    ysq1 = scr.tile([64, N], f32, tag="scr1")
    nc.scalar.activation(ysq1.bitcast(f32r), yT1, Act.Square)
    ocs = ctile([128, 1], f32, "ocs")
    nc.vector.memset(ocs, 1.0)
    onescol0 = ctile([128, 1], f32r, "oc0")
    onescol1 = ctile([64, 1], f32r, "oc1")
    nc.vector.tensor_copy(onescol0, ocs)
    nc.vector.tensor_copy(onescol1, ocs[0:64, :])
    for nh in range(2):
        ps = pt.tile([1, 512], f32, tag="t")
        nc.tensor.matmul(ps, onescol0, ysq0[:, nh * 512:(nh + 1) * 512].bitcast(f32r),
                         start=True, stop=False)
        nc.tensor.matmul(ps, onescol1, ysq1[:, nh * 512:(nh + 1) * 512].bitcast(f32r),
                         start=False, stop=True)
        nc.scalar.activation(y1m[64:65, nh * 512:(nh + 1) * 512].bitcast(f32r),
                             ps, Act.Copy, scale=-0.5)

    # ---- P3: Sp, top-16 threshold, additive mask ----
    maskneg = [ctile([128, N], bf16, f"mn{dt}") for dt in range(4)]
    for dt in range(4):
        S_sb = scr.tile([128, N], f32, tag="S")
        for nh in range(2):
            ps = pt.tile([128, 512], f32, tag="t")
            nc.tensor.matmul(ps, yT0[:, dt * 128:(dt + 1) * 128].bitcast(f32r),
                             yT0[:, nh * 512:(nh + 1) * 512].bitcast(f32r),
                             start=True, stop=False)
            nc.tensor.matmul(ps, y1e[:, dt * 128:(dt + 1) * 128].bitcast(f32r),
                             y1m[:, nh * 512:(nh + 1) * 512].bitcast(f32r),
                             start=False, stop=True)
            nc.scalar.copy(S_sb[:, nh * 512:(nh + 1) * 512], ps)
        m8a = m8p.tile([128, 8], f32, tag="m8a")
        nc.vector.max(out=m8a, in_=S_sb)
        S_rep = scr.tile([128, N], f32, tag="srep")
        nc.vector.match_replace(out=S_rep, in_to_replace=m8a, in_values=S_sb,
                                imm_value=REPL_NEG)
        m8b = m8p.tile([128, 8], f32, tag="m8b")
        nc.vector.max(out=m8b, in_=S_rep)
        nc.vector.tensor_scalar(out=maskneg[dt], in0=S_sb, scalar1=m8b[:, 7:8],
                                scalar2=MASK_NEG, op0=Alu.is_lt, op1=Alu.mult)

    # ---- P4: a_dst columns [dest, 4] and broadcast a_src planes ----
    ad_sb = ctile([128, 4, 4], f32, "ad_sb")
    for mc in range(4):   # only dest chunks need a_dst
        ps = pt.tile([128, 4], f32, tag="t")
        nc.tensor.matmul(ps, yT0[:, mc * 128:(mc + 1) * 128].bitcast(f32r),
                         V0[:, 4:8], start=True, stop=False)
        nc.tensor.matmul(ps, y1e[0:64, mc * 128:(mc + 1) * 128].bitcast(f32r),
                         V1[:, 4:8], start=False, stop=True)
        nc.vector.tensor_copy(ad_sb[:, mc, :], ps)
    asrcB = []
    for hh in range(4):
        row = ctile([1, N], bf16, f"asrcT{hh}")
        for nh2 in range(2):
            ps = pt.tile([1, 512], f32, tag="t", name=f"psat{hh}_{nh2}")
            nc.tensor.matmul(ps, V0[:, hh:hh + 1],
                             yT0[:, nh2 * 512:(nh2 + 1) * 512].bitcast(f32r),
                             start=True, stop=False)
            nc.tensor.matmul(ps, V1[:, hh:hh + 1],
                             y1e[0:64, nh2 * 512:(nh2 + 1) * 512].bitcast(f32r),
                             start=False, stop=True)
            nc.scalar.copy(row[:, nh2 * 512:(nh2 + 1) * 512], ps)
        nc.sync.dma_start(t["ascr"][hh:hh + 1, :], row)
        ab = ctile([128, N], bf16, f"asrcB{hh}")
        bcast = bass.AP(tensor=t["ascr"].tensor, offset=hh * N,
                        ap=[[0, 128], [1, N]])
        nc.sync.dma_start(ab, bcast)
        asrcB.append(ab)

    # ---- P5: h' = y @ Wg' in bf16, [m, head, 385] with 4.0 in col 384 ----
    h_sb = [ctile([128, HEADS, HD + 1], bf16, f"h{mc}") for mc in range(8)]
    for mc in range(8):
        nc.vector.memset(h_sb[mc][:, :, HD:HD + 1], 4.0)
        for hh in range(4):
            ps = pg.tile([128, HD], f32, tag="g", name=f"psh{mc}_{hh}")
            nc.tensor.matmul(ps, yb0[:, mc * 128:(mc + 1) * 128],
                             Wg0[:, hh * HD:(hh + 1) * HD], start=True, stop=False)
            nc.tensor.matmul(ps, yb1[:, mc * 128:(mc + 1) * 128],
                             Wg1[:, hh * HD:(hh + 1) * HD], start=False, stop=True)
            nc.scalar.copy(h_sb[mc][:, hh, 0:HD], ps)

    # ---- CST broadcast [128, 384] ----
    CST = ctile([128, HD], f32, "CST")
    ps_c = pt.tile([128, HD], f32, tag="t")
    nc.tensor.matmul(ps_c, ones_row[:, 0:128], cst_row, start=True, stop=True)
    nc.scalar.copy(CST, ps_c)

    # ---- P6: dest-tile-major attention + aggregation ----
    # Each dest tile dt starts as soon as its own top-16 mask is ready.
    mhp = ctx.enter_context(tc.tile_pool(name="mhp", bufs=4))
    gacc = [ctile([128, HD], f32, f"gacc{dt}") for dt in range(4)]
    for dt in range(4):
        for hh in range(4):
            m_ = mhp.tile([128, N], bf16, tag="mh", name=f"mh{dt}_{hh}")
            nc.vector.scalar_tensor_tensor(out=m_, in0=maskneg[dt],
                                           scalar=ad_sb[:, dt, hh:hh + 1],
                                           in1=asrcB[hh], op0=Alu.add, op1=Alu.add)
            psq = pt.tile([128, N], bf16, tag="t", name=f"psq{dt}_{hh}")
            for mc in range(8):
                nc.tensor.matmul(psq[:, mc * 128:(mc + 1) * 128],
                                 m_[:, mc * 128:(mc + 1) * 128], identb,
                                 is_transpose=True, start=True, stop=True,
                                 skip_group_check=True)
            lr2 = ep.tile([128, N], bf16, tag="lr", name=f"lr{dt}_{hh}")
            nc.scalar.activation(lr2, psq, Act.Prelu, scale=1.0, alpha=0.2)
            wm2 = ep.tile([128, N], bf16, tag="wm", name=f"wm{dt}_{hh}")
            nc.scalar.activation(wm2, lr2, Act.Exp)
            psg = pg.tile([128, HD + 1], f32, tag="g", name=f"psg{dt}_{hh}")
            for mc in range(8):
                nc.tensor.matmul(psg, wm2[:, mc * 128:(mc + 1) * 128],
                                 h_sb[mc][:, hh, :],
                                 start=(mc == 0), stop=(mc == 7))
            rz = rzp.tile([128, 1], f32, tag="rz")
            nc.vector.reciprocal(rz, psg[:, HD:HD + 1])
            src1 = CST if hh == 0 else gacc[dt]
            nc.vector.scalar_tensor_tensor(out=gacc[dt], in0=psg[:, 0:HD],
                                           scalar=rz, in1=src1,
                                           op0=Alu.mult, op1=Alu.add)

    # ---- P7: gelu + transpose to [d, dest] bf16 ----
    g2 = [ctile([128, HD], bf16, f"g2{dt}") for dt in range(4)]
    for dt in range(4):
        nc.scalar.activation(g2[dt], gacc[dt], Act.Gelu)
    g2T = [ctile([128, 512], bf16, f"g2T{dc}") for dc in range(3)]
    for dc in range(3):
        ps = pt.tile([128, 512], bf16, tag="t")
        for dt in range(4):
            nc.tensor.matmul(ps[:, dt * 128:(dt + 1) * 128],
                             g2[dt][:, dc * 128:(dc + 1) * 128], identb,
                             is_transpose=True, start=True, stop=True,
                             skip_group_check=True)
        nc.vector.tensor_copy(g2T[dc], ps)

    # ---- P8: delta = g2 @ W2f.T + b2f  (fp16; host adds the x residual) ----
    f16 = mybir.dt.float16
    for ct, (c0, cl) in enumerate(_CC):
        ps = pt.tile([cl, 512], f32, tag="t")
        for dc in range(3):
            nc.tensor.matmul(ps, Wt[dc][:, c0:c0 + cl], g2T[dc],
                             start=(dc == 0), stop=(dc == 2))
        outT = scr.tile([cl, 512], f16, tag=f"outT{ct}")
        bf = (b2f0, b2f1)[ct]
        nc.vector.tensor_scalar_add(outT, ps, bf)
        nc.sync.dma_start(t["out"][c0:c0 + cl, :], outT)

    if DEBUG_TAPS:
        nc.sync.dma_start(t["d_yT0"], yT0)
        nc.sync.dma_start(t["d_yT1"], yT1)
        nc.sync.dma_start(t["d_msq"], y1m[64:65, :])
        for dt in range(4):
            nc.sync.dma_start(t[f"d_mn{dt}"], maskneg[dt])
            nc.sync.dma_start(t[f"d_gacc{dt}"], gacc[dt])
            nc.sync.dma_start(t[f"d_g2{dt}"], g2[dt])
        nc.sync.dma_start(t["d_a"], a_sb)
        for mc in range(8):
            nc.sync.dma_start(t[f"d_h{mc}"], h_sb[mc])
    ctx.close()


def _build_nc():
    import concourse.bacc as bacc
    import concourse.mybir as mybir
    import concourse.tile as tile

    f32 = mybir.dt.float32
    bf16 = mybir.dt.bfloat16
    nc = bacc.Bacc("TRN2", target_bir_lowering=False, debug=False,
                   enable_asserts=False, num_devices=NCORES)
    t = {}

    def din(name, shape, dt):
        t[name] = nc.dram_tensor(name, shape, dt, kind="ExternalInput").ap()

    din("x", [C, N], f32)
    din("W1fT", [C, C], f32)
    din("b1f", [C, 1], f32)
    din("V", [C, 8], f32)
    din("WgTs", [C, HEADS * HD], bf16)
    din("W2fT", [HD, C], bf16)
    din("b2f", [C, 1], f32)
    din("cst", [1, HD], f32)
    t["out"] = nc.dram_tensor("out", [C, ND], mybir.dt.float16,
                              kind="ExternalOutput").ap()
    t["ascr"] = nc.dram_tensor("ascr", [HEADS, N], bf16, kind="Internal").ap()
    if DEBUG_TAPS:
        def dout(name, shape, dt):
            t[name] = nc.dram_tensor(name, shape, dt, kind="ExternalOutput").ap()
        dout("d_yT0", [128, N], f32)
        dout("d_yT1", [64, N], f32)
        dout("d_msq", [1, N], f32)
        for dt in range(4):
            dout(f"d_mn{dt}", [128, N], bf16)
            dout(f"d_gacc{dt}", [128, HD], f32)
            dout(f"d_g2{dt}", [128, HD], bf16)
        dout("d_a", [128, 8, 8], f32)
        for mc in range(8):
            dout(f"d_h{mc}", [128, HEADS, HD + 1], bf16)

    with tile.TileContext(nc) as tc:
        _emit(nc, tc, t)
    nc.compile()
    return nc


def _host_weights(W1, b1, bn1, Wg, att_src, att_dst, bg, bng, W2, b2, bn2):
    import ml_dtypes
    f8 = np.float64
    s1 = (bn1[0] / np.sqrt(bn1[3] + BN_EPS)).astype(f8)
    W1f = W1.astype(f8) * s1[:, None]
    b1f = ((b1.astype(f8) - bn1[2]) * s1 + bn1[1]).astype(np.float32)
    sg = (bng[0] / np.sqrt(bng[3] + BN_EPS)).astype(f8)
    Wgs = Wg.astype(f8) * np.tile(sg, HEADS)[None, :]
    cst = ((bg.astype(f8) - bng[2]) * sg + bng[1]).astype(np.float32)
    s2 = (bn2[0] / np.sqrt(bn2[3] + BN_EPS)).astype(f8)
    W2f = W2.astype(f8) * s2[:, None]
    b2f = ((b2.astype(f8) - bn2[2]) * s2 + bn2[1]).astype(np.float32)
    # V[:, h] = sum_d Wg[:, h*HD+d] * att_src[h, d]; V[:, 4+h] likewise att_dst
    Wg3 = Wg.astype(f8).reshape(C, HEADS, HD)
    V = np.concatenate([
        np.einsum("chd,hd->ch", Wg3, att_src.astype(f8)),
        np.einsum("chd,hd->ch", Wg3, att_dst.astype(f8)),
    ], axis=1).astype(np.float32)
    return {
        "W1fT": np.ascontiguousarray(W1f.T).astype(np.float32),
        "b1f": b1f.reshape(C, 1),
        "V": V,
        "WgTs": Wgs.astype(ml_dtypes.bfloat16),
        "W2fT": np.ascontiguousarray(W2f.T).astype(ml_dtypes.bfloat16),
        "b2f": b2f.reshape(C, 1),
        "cst": cst.reshape(1, HD),
    }


_CACHE = {}


def _get_nc():
    if "nc" not in _CACHE:
        _CACHE["nc"] = _build_nc()
    return _CACHE["nc"]


def make_in_maps(x, weights):
    """x: [B, C, H, W] fp32; weights: dict from _host_weights."""
    xs = np.asarray(x, np.float32).reshape(B, C, N)
    in_maps = []
    for core in range(NCORES):
        img = core // 2
        xi = xs[img]
        if core % 2 == 1:
            xi = np.concatenate([xi[:, ND:], xi[:, :ND]], axis=1)
        m = dict(weights)
        m["x"] = np.ascontiguousarray(xi)
        in_maps.append(m)
    return in_maps


def assemble(results, x):
    """results: per-core fp16 delta [C, ND]; adds the fp32 x residual back."""
    out = np.asarray(x, np.float32).reshape(B, C, N).copy()
    for core in range(NCORES):
        img, half = core // 2, core % 2
        out[img][:, half * ND:(half + 1) * ND] += np.asarray(
            results[core]["out"], np.float32)
    return out.reshape(B, C, H, W)


def _get_runner():
    """Build (once) a cached jitted dispatcher for the 8-core NEFF.

    Mirrors concourse.bass2jax.run_bass_via_pjrt's multi-core path, but keeps
    the jitted shard_map callable across kernel() invocations so repeat calls
    pay only dispatch + transfer, not retrace/recompile.
    """
    if "runner" in _CACHE:
        return _CACHE["runner"]

    import jax
    import jax.core
    import concourse.mybir as mybir
    from concourse import bass2jax
    from jax.sharding import Mesh, PartitionSpec
    from jax.experimental.shard_map import shard_map

    nc = _get_nc()
    bass2jax.install_neuronx_cc_hook()

    partition_name = (nc.partition_id_tensor.name
                      if nc.partition_id_tensor is not None else None)
    in_names, out_names, out_avals = [], [], []
    for alloc in nc.m.functions[0].allocations:
        if not isinstance(alloc, mybir.MemoryLocationSet):
            continue
        name = alloc.memorylocations[0].name
        if alloc.kind == "ExternalInput":
            if name != partition_name:
                in_names.append(name)
        elif alloc.kind == "ExternalOutput":
            out_names.append(name)
            out_avals.append(jax.core.ShapedArray(
                tuple(alloc.tensor_shape), mybir.dt.np(alloc.dtype)))
    n_params = len(in_names)
    n_outs = len(out_names)
    all_names = in_names + out_names
    if partition_name is not None:
        all_names = all_names + [partition_name]

    def _body(*args):
        operands = list(args)
        if partition_name is not None:
            operands.append(bass2jax.partition_id_tensor())
        outs = bass2jax._bass_exec_p.bind(
            *operands,
            out_avals=tuple(out_avals),
            in_names=tuple(all_names),
            out_names=tuple(out_names),
            lowering_input_output_aliases=(),
            sim_require_finite=True,
            sim_require_nnan=True,
            nc=nc,
        )
        return tuple(outs)

    devices = jax.devices()[:NCORES]
    mesh = Mesh(np.asarray(devices), ("core",))
    specs = (PartitionSpec("core"),) * (n_params + n_outs)
    # No donation: the kernel writes every element of every output, so the
    # "initial output" operands can be one reusable device-resident buffer.
    sharded = jax.jit(
        shard_map(_body, mesh=mesh, in_specs=specs,
                  out_specs=(PartitionSpec("core"),) * n_outs, check_rep=False),
        keep_unused=True)
    import jax as _jax
    dzero = [_jax.device_put(np.zeros((NCORES * av.shape[0], *av.shape[1:]),
                                      av.dtype)) for av in out_avals]

    _CACHE["runner"] = (sharded, in_names, out_names, out_avals, dzero)
    return _CACHE["runner"]


def _digest(arrs):
    import hashlib
    h = hashlib.blake2b(digest_size=16)
    for a in arrs:
        a = np.asarray(a)
        h.update(a.tobytes())
    return h.hexdigest()


def run_in_maps(in_maps):
    """Run the prebuilt NEFF on 8 cores; returns list of per-core out dicts."""
    import jax
    sharded, in_names, out_names, out_avals, dzero = _get_runner()
    concat_in = [
        np.concatenate([np.asarray(in_maps[c][nm]) for c in range(NCORES)], axis=0)
        for nm in in_names
    ]
    din = [jax.device_put(a) for a in concat_in]
    out_arrs = sharded(*din, *dzero)
    return [
        {nm: np.asarray(out_arrs[i]).reshape(NCORES, *out_avals[i].shape)[c]
         for i, nm in enumerate(out_names)}
        for c in range(NCORES)
    ]


def _device_inputs(x, W1, b1, bn1, Wg, att_src, att_dst, bg, bng, W2, b2, bn2):
    """Device-resident concat inputs, cached on the content of the arguments."""
    import jax
    _, in_names, _, _, _ = _get_runner()
    wkey = _digest([W1, b1, bn1, Wg, att_src, att_dst, bg, bng, W2, b2, bn2])
    if _CACHE.get("wkey") != wkey:
        w = _host_weights(W1, b1, bn1, Wg, att_src, att_dst, bg, bng, W2, b2, bn2)
        dw = {}
        for nm in in_names:
            if nm == "x":
                continue
            rep = np.broadcast_to(
                w[nm], (NCORES, *w[nm].shape)).reshape(NCORES * w[nm].shape[0],
                                                       *w[nm].shape[1:])
            dw[nm] = jax.device_put(np.ascontiguousarray(rep))
        _CACHE["dw"] = dw
        _CACHE["wkey"] = wkey
    xkey = _digest([x])
    if _CACHE.get("xkey") != xkey:
        xs = np.asarray(x, np.float32).reshape(B, C, N)
        xsh = np.empty((NCORES, C, N), np.float32)
        for core in range(NCORES):
            img, half = core // 2, core % 2
            if half == 0:
                xsh[core] = xs[img]
            else:
                xsh[core][:, 0:ND] = xs[img][:, ND:]
                xsh[core][:, ND:] = xs[img][:, 0:ND]
        _CACHE["dx"] = jax.device_put(xsh.reshape(NCORES * C, N))
        _CACHE["xkey"] = xkey
    return [_CACHE["dx"] if nm == "x" else _CACHE["dw"][nm] for nm in in_names]


def kernel(x, W1, b1, bn1, Wg, att_src, att_dst, bg, bng, W2, b2, bn2):
    sharded, in_names, out_names, out_avals, dzero = _get_runner()
    din = _device_inputs(x, W1, b1, bn1, Wg, att_src, att_dst, bg, bng,
                         W2, b2, bn2)
    try:
        out_arrs = sharded(*din, *dzero)
        delta = np.asarray(out_arrs[out_names.index("out")])
    except Exception:
        # transient device faults (e.g. NRT_EXEC_UNIT_UNRECOVERABLE) have
        # been observed on this tunnel; retry once with re-uploaded inputs
        import time as _time
        _time.sleep(0.5)
        _CACHE.pop("xkey", None)
        _CACHE.pop("wkey", None)
        din = _device_inputs(x, W1, b1, bn1, Wg, att_src, att_dst, bg, bng,
                             W2, b2, bn2)
        out_arrs = sharded(*din, *dzero)
        delta = np.asarray(out_arrs[out_names.index("out")])
    results = [{"out": delta.reshape(NCORES, C, ND)[c]} for c in range(NCORES)]
    return assemble(results, x)
